# revision 6
# baseline (speedup 1.0000x reference)
"""Trainium2 Bass kernel for EnhancedOFTQKVLayer.

Computes out[b,s,o] = x[b,s,:] @ filt[o,:]^T + bias[o] where
filt = [Wq @ BD(cayley(q_R)); Wk @ BD(cayley(k_R)); Wv @ BD(cayley(v_R))]
(BD = block-diagonal, cayley(A) = (I-S) inv(I+S+eps I), S = 0.5(A-A^T)).

Distribution: data-parallel - batch b (8 rows) sharded one per NeuronCore;
attn_weight / bias / rotation blocks replicated.

Per-core schedule (v4):
  1. Cayley via SPD Newton-Schulz on P = (1+eps)^2 I - S^2, all 6 sets of
     4 blocks interleaved (chain-latency hiding), fp16 iterations + fp32
     polish.  The q-projection sets are polished first so the main GEMM
     can start while k/v sets finish.  rmat is pre-permuted on the host to
     [128, 24, 128] so one contiguous DMA loads all rotation blocks.
  2. Early-deadline operand tiles (W^T og0/og1, x^T sg0/sg1) are built on
     the PE with fp32 transpose-mode + cast-on-PSUM-copy - this also keeps
     the PE busy (HAM-warm) through the Newton chains.  Everything else
     (W^T og2-5, x^T sg2-7) is cast fp32->bf16 in DRAM by SWDGE cast-DMAs
     and pulled into SBUF pre-transposed by 1 MB DMA-xbar loads: zero
     compute-engine cost, soft deadlines.
  3. Main matmul in bf16 (fp32 PSUM), three phases: (1) og0 then og1 over
     the first 8 row tiles, interleaved with the k/v Newton polish and the
     remaining filtT builds, (2) row-tile-outer over the remaining 24
     tiles x all 6 og groups (x^T arrives through a 2-slot ring),
     (3) og2-5 for the first 8 row tiles (pure GEMM tail; their x^T tiles
     stay resident).  Fused bias add on DVE, 256-512 KB output DMAs on
     the ACT queue.
"""

import numpy as np

import concourse.bass as bass
import concourse.mybir as mybir
import concourse.tile as tile
from concourse import bacc
from concourse.bass import ds, ts
from concourse.masks import make_identity
from concourse.bass_utils import run_bass_kernel_spmd

F32 = mybir.dt.float32
F16 = mybir.dt.float16
BF16 = mybir.dt.bfloat16

MAIN_DT = BF16           # dtype of the big matmul inputs (x, filtT)

HIDDEN = 1024
OUT_DIM = 3 * HIDDEN
SEQ = 4096
P = 128
NBLK = 8                 # 128-blocks per hidden
NROT = 24                # 3 * NBLK rotation blocks
EPS = 1e-6
N_CORES = 8

NSETS = 6                # Newton processes blocks in sets of 4
SETB = 4

# Newton-Schulz schedule (validated offline against the jax reference).
NEWTON_F16 = 7
SYM_ITERS = {3, 5}       # symmetrize on these fp16 iterations
X0_A = 0.0152174         # X0 = aI + bP (degree-1 minimax init on [1, 260])
X0_B = -5.78922e-05

M_TILES = SEQ // P       # 32 row tiles of 128
SG = SEQ // 512          # 8 row groups of 512 (4 row tiles each)
O_TILES = OUT_DIM // 512  # 6
N1 = 8                   # phase-1 row tiles (og0/og1 early wave)


def build_body(ctx, tc):
    nc = tc.nc

    x = nc.dram_tensor("x", [SEQ, HIDDEN], F32, kind="ExternalInput").ap()
    w = nc.dram_tensor("w", [OUT_DIM, HIDDEN], F32, kind="ExternalInput").ap()
    bias = nc.dram_tensor("bias", [OUT_DIM], F32, kind="ExternalInput").ap()
    # host-side pre-permuted rotations: rmt[p, n, c] = rmat[n, p, c]
    rmt_d = nc.dram_tensor("rmt", [P, NROT, P], F32, kind="ExternalInput").ap()
    out = nc.dram_tensor("out", [SEQ, OUT_DIM], F32, kind="ExternalOutput").ap()

    sub = nc.vector.tensor_sub
    add = nc.vector.tensor_add
    smul = nc.vector.tensor_scalar_mul
    cp = nc.vector.tensor_copy
    scp = nc.scalar.copy

    def bc(t):  # broadcast a [P, P] constant over a set's middle dim
        return t[:].unsqueeze(1).to_broadcast([P, SETB, P])

    # ---- persistent pools ----
    const = ctx.enter_context(tc.tile_pool(name="const", bufs=1))
    ftp = ctx.enter_context(tc.tile_pool(name="ftp", bufs=1))
    qpool = ctx.enter_context(tc.tile_pool(name="qpool", bufs=1))
    dram = ctx.enter_context(tc.tile_pool(name="dram", bufs=1, space="DRAM"))

    ident32 = const.tile([P, P], F32)
    make_identity(nc, ident32)
    eI2 = const.tile([P, P], F32)       # (1+eps)^2 I
    smul(eI2[:], ident32[:], float((1.0 + EPS) ** 2))
    eI12 = const.tile([P, P], F32)      # ((1+eps) + (1+eps)^2) I
    smul(eI12[:], ident32[:], float((1.0 + EPS) + (1.0 + EPS) ** 2))
    twoI = const.tile([P, P], F32)      # 2 I
    smul(twoI[:], ident32[:], 2.0)
    aI0 = const.tile([P, P], F16)       # X0_A * I  (Newton init)
    smul(aI0[:], ident32[:], float(X0_A))

    # bf16 DRAM scratch (filled by SWDGE cast-DMAs; og2-5 / sg2-7 only)
    xb = dram.tile([SEQ, HIDDEN], MAIN_DT)
    wb = dram.tile([OUT_DIM, HIDDEN], MAIN_DT)

    # gpsimd queue: W og2-5 casts, x sg2/3 casts, bias broadcast, rest.
    bias_bc = const.tile([P, OUT_DIM], MAIN_DT)
    with tc.tile_pool(name="biasld", bufs=1) as bl:
        brow = bl.tile([1, OUT_DIM], F32)
        nc.sync.dma_start(brow[:], bias.unsqueeze(0))
        cp(bias_bc[:1, :], brow[:])

    for og in range(2, O_TILES):
        nc.gpsimd.dma_start(wb[ts(og, 512), :], w[ts(og, 512), :])
    for sg in (2, 3):
        nc.gpsimd.dma_start(xb[ts(sg, 512), :], x[ts(sg, 512), :])
    nc.gpsimd.partition_broadcast(bias_bc[:], bias_bc[:1, :])
    for sg in range(4, SG):
        nc.gpsimd.dma_start(xb[ts(sg, 512), :], x[ts(sg, 512), :])

    # filtT chunks: ft[k][og][c, o'] = filtT[k*128+c, og*512+o']
    ft = [[ftp.tile([P, 512], MAIN_DT, tag=f"ft{k}_{og}", name=f"ft{k}_{og}")
           for og in range(O_TILES)] for k in range(NBLK)]

    with (
        tc.tile_pool(name="nper", bufs=1) as nper,     # per-set persistents
        tc.tile_pool(name="nx", bufs=1) as nxp,        # per-set iterates
        tc.tile_pool(name="nu", bufs=3) as nup,        # U temp
        tc.tile_pool(name="misc", bufs=1) as misc,
        tc.tile_pool(name="wld", bufs=2) as wld,       # W fp32 rows (og0/1)
        tc.tile_pool(name="xld", bufs=2) as xld,       # x fp32 rows (sg0/1)
        tc.tile_pool(name="wtsp", bufs=2) as wtsp,     # W^T og ring
        tc.tile_pool(name="xt01", bufs=1) as xt01p,    # x^T sg0/sg1 resident
        tc.tile_pool(name="xtp", bufs=2) as xtp,       # x^T sg2-7 ring
        tc.tile_pool(name="obp", bufs=3) as obp,       # out staging
        tc.tile_pool(name="ps_g", bufs=4, space="PSUM") as ps_g,
        tc.tile_pool(name="ps_out", bufs=4, space="PSUM") as ps_out,
    ):
        # ------- rotations + S prep (rmt freed right after) -------
        s_s = []
        with tc.tile_pool(name="rmt", bufs=1) as rmtp:
            rmt = rmtp.tile([P, NROT, P], F32)
            nc.sync.dma_start(rmt[:], rmt_d)
            for s in range(NSETS):
                tpg = ps_g.tile([P, SETB, P], F32, tag="g")
                for j in range(SETB):
                    nc.tensor.transpose(tpg[:, j, :], rmt[:, s * SETB + j, :],
                                        ident32[:])
                sset = nper.tile([P, SETB, P], F32, tag=f"s{s}", name=f"s{s}")
                for j in range(SETB):
                    sub(sset[:, j, :], rmt[:, s * SETB + j, :], tpg[:, j, :])
                smul(sset[:], sset[:], 0.5)              # S
                s_s.append(sset)

        # ------- W^T og0/og1 and x^T sg0/sg1 on the PE (early + warm) ----
        wts = {}

        def emit_wT_pe(og):
            wts[og] = wtsp.tile([P, NBLK, 512], MAIN_DT, tag="wts",
                                name=f"wts{og}")
            for j4 in range(4):
                ot = og * 4 + j4
                wrow = wld.tile([P, HIDDEN], F32, tag="wrow", name=f"wr{ot}")
                nc.sync.dma_start(wrow[:], w[ts(ot, P), :])
                for kh in range(2):
                    tpg = ps_g.tile([P, SETB, P], F32, tag="g")
                    for k4 in range(SETB):
                        nc.tensor.transpose(tpg[:, k4, :],
                                            wrow[:, ts(kh * SETB + k4, P)],
                                            ident32[:])
                    dst = wts[og][:, ts(kh, SETB), ts(j4, P)]
                    if (j4 + kh) % 2 == 0:
                        cp(dst, tpg[:])
                    else:
                        scp(dst, tpg[:])

        def emit_wT_xbar(og):
            wts[og] = wtsp.tile([P, NBLK, 512], MAIN_DT, tag="wts",
                                name=f"wts{og}")
            nc.sync.dma_start(wts[og][:], wb[ts(og, 512), :], transpose=True)

        xts = {}

        def emit_xt_pe(sg):
            t = xt01p.tile([P, NBLK, 512], MAIN_DT, tag=f"xt{sg}",
                           name=f"xt{sg}")
            for sb in range(4):
                mt = sg * 4 + sb
                xr = xld.tile([P, HIDDEN], F32, tag="xr", name=f"xr{mt}")
                nc.sync.dma_start(xr[:], x[ts(mt, P), :])
                for kh in range(2):
                    tpg = ps_g.tile([P, SETB, P], F32, tag="g")
                    for k4 in range(SETB):
                        nc.tensor.transpose(tpg[:, k4, :],
                                            xr[:, ts(kh * SETB + k4, P)],
                                            ident32[:])
                    dst = t[:, ts(kh, SETB), ts(sb, P)]
                    if (sb + kh) % 2 == 0:
                        cp(dst, tpg[:])
                    else:
                        scp(dst, tpg[:])
            xts[sg] = t

        def emit_xt_xbar(sg):
            t = xtp.tile([P, NBLK, 512], MAIN_DT, tag="xt", name=f"xt{sg}")
            nc.sync.dma_start(t[:], xb[ts(sg, 512), :], transpose=True)
            xts[sg] = t

        emit_wT_pe(0)
        emit_wT_pe(1)
        emit_xt_pe(0)
        emit_xt_pe(1)

        # ---------- Newton-Cayley: 6 interleaved sets of 4 blocks ----------
        p32_s, p16_s, x_s = [], [], []
        for s in range(NSETS):
            g = ps_g.tile([P, SETB, P], F32, tag="g")
            for j in range(SETB):                        # S^T @ S = -S^2
                nc.tensor.matmul(g[:, j, :], lhsT=s_s[s][:, j, :],
                                 rhs=s_s[s][:, j, :], start=True, stop=True)
            p32s = nper.tile([P, SETB, P], F32, tag=f"p32{s}", name=f"p32{s}")
            add(p32s[:], bc(eI2), g[:])                  # P = (1+e)^2 I - S^2
            p16s = nper.tile([P, SETB, P], F16, tag=f"p16{s}", name=f"p16{s}")
            scp(p16s[:], p32s[:])
            xset = nxp.tile([P, SETB, P], F16, tag=f"x{s}", name=f"x{s}_init")
            smul(xset[:], p32s[:], float(X0_B))          # X0 = aI + bP
            add(xset[:], xset[:], bc(aI0))
            # fold B^T = eI12 + (2+e)S - P into the S tile now
            nc.vector.tensor_scalar(s_s[s][:], s_s[s][:], float(2.0 + EPS),
                                    None, mybir.AluOpType.mult)
            add(s_s[s][:], s_s[s][:], bc(eI12))
            sub(s_s[s][:], s_s[s][:], p32s[:])
            p32_s.append(p32s)
            p16_s.append(p16s)
            x_s.append(xset)

        for i in range(NEWTON_F16):
            do_sym = i in SYM_ITERS
            for s in range(NSETS):
                # one PSUM tile per (iter, set): g1, then g2 overwrites it
                # (the DVE read of g1 gates the g2 matmuls anyway).
                g = ps_g.tile([P, SETB, P], F32, tag="g")
                for j in range(SETB):
                    nc.tensor.matmul(g[:, j, :], lhsT=p16_s[s][:, j, :],
                                     rhs=x_s[s][:, j, :], start=True,
                                     stop=True)
                u = nup.tile([P, SETB, P], F16, tag="u")
                sub(u[:], bc(twoI), g[:])                # U = 2I - P X (DVE)
                for j in range(SETB):                    # X' = X U
                    nc.tensor.matmul(g[:, j, :], lhsT=x_s[s][:, j, :],
                                     rhs=u[:, j, :], start=True, stop=True)
                xset = nxp.tile([P, SETB, P], F16, tag=f"x{s}",
                                name=f"x{s}_{i}")
                if not do_sym:
                    if s == 0:
                        cp(xset[:], g[:])                # DVE
                    else:
                        scp(xset[:], g[:])               # ScalarE
                else:
                    xc = misc.tile([P, SETB, P], F32, tag="xc")
                    cp(xc[:], g[:])
                    tpg = ps_g.tile([P, SETB, P], F32, tag="g")
                    for j in range(SETB):
                        nc.tensor.transpose(tpg[:, j, :], xc[:, j, :],
                                            ident32[:])
                    add(xc[:], xc[:], tpg[:])
                    nc.scalar.activation(xset[:], xc[:],
                                         mybir.ActivationFunctionType.Copy,
                                         scale=0.5)
                x_s[s] = xset

        # fp32 polish + Q for a pair of sets (q first, then k, then v)
        q_s = [None] * NSETS

        def polish_q(pair):
            xfs = {}
            for s in pair:
                xf = nxp.tile([P, SETB, P], F32, tag=f"xf{s % 2}",
                              name=f"xf{s}")
                if s % 2 == 0:
                    cp(xf[:], x_s[s][:])
                else:
                    scp(xf[:], x_s[s][:])
                xfs[s] = xf
            for s in pair:
                g = ps_g.tile([P, SETB, P], F32, tag="g")
                for j in range(SETB):
                    nc.tensor.matmul(g[:, j, :], lhsT=p32_s[s][:, j, :],
                                     rhs=xfs[s][:, j, :], start=True,
                                     stop=True)
                uf = misc.tile([P, SETB, P], F32, tag=f"uf{s % 2}")
                sub(uf[:], bc(twoI), g[:])
                for j in range(SETB):
                    nc.tensor.matmul(g[:, j, :], lhsT=xfs[s][:, j, :],
                                     rhs=uf[:, j, :], start=True, stop=True)
                xf2 = nxp.tile([P, SETB, P], F32, tag=f"xf{s % 2}",
                               name=f"xf2_{s}")
                if s % 2 == 0:
                    cp(xf2[:], g[:])
                else:
                    scp(xf2[:], g[:])
                xfs[s] = xf2
            for s in pair:
                # Q = B @ X with B^T = eI12 + (2+e)S - P (folded into s_s)
                g = ps_g.tile([P, SETB, P], F32, tag="g")
                for j in range(SETB):
                    nc.tensor.matmul(g[:, j, :], lhsT=s_s[s][:, j, :],
                                     rhs=xfs[s][:, j, :], start=True,
                                     stop=True)
                qset = qpool.tile([P, SETB, P], MAIN_DT, tag=f"q{s}",
                                  name=f"q{s}")
                if s % 2 == 0:
                    cp(qset[:], g[:])
                else:
                    scp(qset[:], g[:])
                q_s[s] = qset

        def q_lhsT(n):
            return q_s[n // SETB][:, n % SETB, :]

        # ---- filtT chunks: ft[k][og] = Q^T W^T ----
        def emit_ft(og):
            part = og // 2             # q/k/v
            for k in range(NBLK):
                fg = ps_out.tile([P, 512], F32, tag="po", name=f"fg{og}_{k}")
                nc.tensor.matmul(fg[:], lhsT=q_lhsT(part * NBLK + k),
                                 rhs=wts[og][:, k, :], start=True, stop=True)
                if k % 2 == 0:
                    cp(ft[k][og][:], fg[:])
                else:
                    scp(ft[k][og][:], fg[:])

        # ---- main GEMM bursts ----
        def emit_gemm(mt, og_lo, n_og):
            sg, sb = mt // 4, mt % 4
            ob = obp.tile([P, 512 * n_og], F32, tag="ob",
                          name=f"ob{mt}_{og_lo}")
            for h in range(n_og):
                og = og_lo + h
                po = ps_out.tile([P, 512], F32, tag="po", name=f"po{mt}_{og}")
                for k in range(NBLK):
                    nc.tensor.matmul(po[:], lhsT=xts[sg][:, k, ts(sb, P)],
                                     rhs=ft[k][og][:],
                                     start=(k == 0), stop=(k == NBLK - 1))
                add(ob[:, ts(h, 512)], po[:], bias_bc[:, ts(og, 512)])
            nc.scalar.dma_start(
                out[ts(mt, P), ds(og_lo * 512, 512 * n_og)], ob[:])

        # ================= tail emission order (= priority) =============
        polish_q((0, 1))
        emit_ft(0)
        emit_ft(1)
        # phase 1: og0 then og1 over the first N1 row tiles; k/v polish and
        # og2-5 filtT builds slot into the stream between chunks.
        for mt in range(N1):
            emit_gemm(mt, 0, 1)
            if mt == 0:
                polish_q((2, 3))
            if mt == 2:
                emit_wT_xbar(2)
                emit_ft(2)
            if mt == 4:
                emit_wT_xbar(3)
                emit_ft(3)
        for mt in range(N1):
            emit_gemm(mt, 1, 1)
            if mt == 0:
                polish_q((4, 5))
            if mt == 2:
                emit_wT_xbar(4)
                emit_ft(4)
            if mt == 4:
                emit_wT_xbar(5)
                emit_ft(5)
        emit_xt_xbar(2)
        emit_xt_xbar(3)
        # phase 2: remaining row tiles, all og groups
        for mt in range(N1, M_TILES):
            if mt % 4 == 0 and mt + 8 < M_TILES:
                emit_xt_xbar((mt + 8) // 4)  # prefetch sg two groups ahead
            for pair in range(3):
                emit_gemm(mt, 2 * pair, 2)
        # phase 3: og2-5 for the first N1 row tiles (x^T still resident)
        for mt in range(N1):
            emit_gemm(mt, 2, 2)
            emit_gemm(mt, 4, 2)


def build():
    if "nc" in _CACHE:
        return _CACHE["nc"]
    import contextlib

    nc = bacc.Bacc("TRN2", target_bir_lowering=False, debug=False)
    with tile.TileContext(nc) as tc:
        with contextlib.ExitStack() as ctx:
            build_body(ctx, tc)
    nc.compile()
    _CACHE["nc"] = nc
    return nc


_CACHE = {}


def make_in_maps(attn_weight, bias, x, q_R, k_R, v_R):
    rmat = np.concatenate([q_R, k_R, v_R], axis=0).astype(np.float32)
    rmt = np.ascontiguousarray(rmat.transpose(1, 0, 2))  # [P, NROT, P]
    w = np.ascontiguousarray(attn_weight, dtype=np.float32)
    b = np.ascontiguousarray(bias, dtype=np.float32)
    return [
        {"x": np.ascontiguousarray(x[c], dtype=np.float32),
         "w": w, "bias": b, "rmt": rmt}
        for c in range(N_CORES)
    ]


def kernel(attn_weight, bias, x, q_R, k_R, v_R, **run_kwargs):
    nc = build()
    in_maps = make_in_maps(attn_weight, bias, x, q_R, k_R, v_R)
    res = run_bass_kernel_spmd(nc, in_maps, core_ids=list(range(N_CORES)),
                               **run_kwargs)
    out = np.stack([res.results[c]["out"] for c in range(N_CORES)], axis=0)
    _CACHE["last_results"] = res
    return out


# revision 11
# speedup vs baseline: 1.0962x; 1.0962x over previous
"""Trainium2 Bass kernel for EnhancedOFTQKVLayer.

Computes out[b,s,o] = x[b,s,:] @ filt[o,:]^T + bias[o] where
filt = [Wq @ BD(cayley(q_R)); Wk @ BD(cayley(k_R)); Wv @ BD(cayley(v_R))]
(BD = block-diagonal, cayley(A) = (I-S) inv(I+S+eps I), S = 0.5(A-A^T)).

Distribution: data-parallel - batch b (8 rows) sharded one per NeuronCore;
attn_weight / bias / rotation blocks replicated.

Per-core schedule (v4):
  1. Cayley via SPD Newton-Schulz on P = (1+eps)^2 I - S^2, all 6 sets of
     4 blocks interleaved (chain-latency hiding), fp16 iterations + fp32
     polish.  The q-projection sets are polished first so the main GEMM
     can start while k/v sets finish.  rmat is pre-permuted on the host to
     [128, 24, 128] so one contiguous DMA loads all rotation blocks.
  2. Early-deadline operand tiles (W^T og0/og1, x^T sg0/sg1) are built on
     the PE with fp32 transpose-mode + cast-on-PSUM-copy - this also keeps
     the PE busy (HAM-warm) through the Newton chains.  Everything else
     (W^T og2-5, x^T sg2-7) is cast fp32->bf16 in DRAM by SWDGE cast-DMAs
     and pulled into SBUF pre-transposed by 1 MB DMA-xbar loads: zero
     compute-engine cost, soft deadlines.
  3. Main matmul in bf16 (fp32 PSUM), three phases: (1) og0 then og1 over
     the first 8 row tiles, interleaved with the k/v Newton polish and the
     remaining filtT builds, (2) row-tile-outer over the remaining 24
     tiles x all 6 og groups (x^T arrives through a 2-slot ring),
     (3) og2-5 for the first 8 row tiles (pure GEMM tail; their x^T tiles
     stay resident).  Fused bias add on DVE, 256-512 KB output DMAs on
     the ACT queue.
"""

import numpy as np

import concourse.bass as bass
import concourse.mybir as mybir
import concourse.tile as tile
from concourse import bacc
from concourse.bass import ds, ts
from concourse.masks import make_identity
from concourse.bass_utils import run_bass_kernel_spmd

F32 = mybir.dt.float32
F16 = mybir.dt.float16
BF16 = mybir.dt.bfloat16

MAIN_DT = BF16           # dtype of the big matmul inputs (x, filtT)

HIDDEN = 1024
OUT_DIM = 3 * HIDDEN
SEQ = 4096
P = 128
NBLK = 8                 # 128-blocks per hidden
NROT = 24                # 3 * NBLK rotation blocks
EPS = 1e-6
N_CORES = 8

NSETS = 6                # Newton processes blocks in sets of 4
SETB = 4

# Newton-Schulz schedule (validated offline against the jax reference).
NEWTON_F16 = 7
SYM_ITERS = {3, 5}       # symmetrize on these fp16 iterations
X0_A = 0.0152174         # X0 = aI + bP (degree-1 minimax init on [1, 260])
X0_B = -5.78922e-05

M_TILES = SEQ // P       # 32 row tiles of 128
SG = SEQ // 512          # 8 row groups of 512 (4 row tiles each)
O_TILES = OUT_DIM // 512  # 6
N1 = 8                   # phase-1 row tiles (og0/og1 early wave)


def build_body(ctx, tc):
    nc = tc.nc

    x = nc.dram_tensor("x", [SEQ, HIDDEN], F32, kind="ExternalInput").ap()
    w = nc.dram_tensor("w", [OUT_DIM, HIDDEN], F32, kind="ExternalInput").ap()
    bias = nc.dram_tensor("bias", [OUT_DIM], F32, kind="ExternalInput").ap()
    # host-side pre-permuted rotations: rmt[p, n, c] = rmat[n, p, c]
    rmt_d = nc.dram_tensor("rmt", [P, NROT, P], F32, kind="ExternalInput").ap()
    out = nc.dram_tensor("out", [SEQ, OUT_DIM], F32, kind="ExternalOutput").ap()

    sub = nc.vector.tensor_sub
    add = nc.vector.tensor_add
    smul = nc.vector.tensor_scalar_mul
    cp = nc.vector.tensor_copy
    scp = nc.scalar.copy

    def bc(t):  # broadcast a [P, P] constant over a set's middle dim
        return t[:].unsqueeze(1).to_broadcast([P, SETB, P])

    # ---- persistent pools ----
    const = ctx.enter_context(tc.tile_pool(name="const", bufs=1))
    ftp = ctx.enter_context(tc.tile_pool(name="ftp", bufs=1))
    qpool = ctx.enter_context(tc.tile_pool(name="qpool", bufs=1))
    dram = ctx.enter_context(tc.tile_pool(name="dram", bufs=1, space="DRAM"))

    ident32 = const.tile([P, P], F32)
    make_identity(nc, ident32)
    eI2 = const.tile([P, P], F32)       # (1+eps)^2 I
    smul(eI2[:], ident32[:], float((1.0 + EPS) ** 2))
    eI12 = const.tile([P, P], F32)      # ((1+eps) + (1+eps)^2) I
    smul(eI12[:], ident32[:], float((1.0 + EPS) + (1.0 + EPS) ** 2))
    twoI = const.tile([P, P], F32)      # 2 I
    smul(twoI[:], ident32[:], 2.0)
    aI0 = const.tile([P, P], F16)       # X0_A * I  (Newton init)
    smul(aI0[:], ident32[:], float(X0_A))

    # bf16 DRAM scratch (filled by SWDGE cast-DMAs; og2-5 / sg2-7 only)
    xb = dram.tile([SEQ, HIDDEN], MAIN_DT)
    wb = dram.tile([OUT_DIM, HIDDEN], MAIN_DT)

    bias_bc = const.tile([P, OUT_DIM], MAIN_DT)
    with tc.tile_pool(name="biasld", bufs=1) as bl:
        brow = bl.tile([1, OUT_DIM], F32)
        nc.sync.dma_start(brow[:], bias.unsqueeze(0))
        cp(bias_bc[:1, :], brow[:])
    nc.gpsimd.partition_broadcast(bias_bc[:], bias_bc[:1, :])

    # filtT chunks: ft[k][og][c, o'] = filtT[k*128+c, og*512+o']
    ft = [[ftp.tile([P, 512], MAIN_DT, tag=f"ft{k}_{og}", name=f"ft{k}_{og}")
           for og in range(O_TILES)] for k in range(NBLK)]

    with (
        tc.tile_pool(name="nper", bufs=1) as nper,     # per-set persistents
        tc.tile_pool(name="nx", bufs=1) as nxp,        # per-set iterates
        tc.tile_pool(name="nu", bufs=3) as nup,        # U temp
        tc.tile_pool(name="misc", bufs=1) as misc,
        tc.tile_pool(name="wld", bufs=2) as wld,       # W fp32 rows (og0/1)
        tc.tile_pool(name="xld", bufs=2) as xld,       # x fp32 rows (sg0/1)
        tc.tile_pool(name="wtsp", bufs=2) as wtsp,     # W^T og ring
        tc.tile_pool(name="xt01", bufs=1) as xt01p,    # x^T sg0/sg1 resident
        tc.tile_pool(name="xtp", bufs=2) as xtp,       # x^T sg2-7 ring
        tc.tile_pool(name="obp", bufs=3) as obp,       # out staging
        tc.tile_pool(name="ps_g", bufs=4, space="PSUM") as ps_g,
        tc.tile_pool(name="ps_out", bufs=4, space="PSUM") as ps_out,
    ):
        # ------- rotations + S prep (rmt freed right after) -------
        s_s = []
        with tc.tile_pool(name="rmt", bufs=1) as rmtp:
            rmt = rmtp.tile([P, NROT, P], F32)
            nc.sync.dma_start(rmt[:], rmt_d)
            for s in range(NSETS):
                tpg = ps_g.tile([P, SETB, P], F32, tag="g")
                for j in range(SETB):
                    nc.tensor.transpose(tpg[:, j, :], rmt[:, s * SETB + j, :],
                                        ident32[:])
                sset = nper.tile([P, SETB, P], F32, tag=f"s{s}", name=f"s{s}")
                for j in range(SETB):
                    sub(sset[:, j, :], rmt[:, s * SETB + j, :], tpg[:, j, :])
                smul(sset[:], sset[:], 0.5)              # S
                s_s.append(sset)

        # ------- W^T og0/og1 and x^T sg0/sg1 on the PE, built one row-tile
        # "unit" at a time so the loads never head-of-line-block Newton;
        # units are interleaved into the fp16 iteration stream below. ----
        wts = {0: wtsp.tile([P, NBLK, 512], MAIN_DT, tag="wts", name="wts0"),
               1: wtsp.tile([P, NBLK, 512], MAIN_DT, tag="wts", name="wts1")}
        xts = {0: xt01p.tile([P, NBLK, 512], MAIN_DT, tag="xt0", name="xt0"),
               1: xt01p.tile([P, NBLK, 512], MAIN_DT, tag="xt1", name="xt1")}

        def emit_unit(kind, grp, j):
            if kind == "w":
                row = wld.tile([P, HIDDEN], F32, tag="wrow",
                               name=f"wr{grp}_{j}")
                nc.sync.dma_start(row[:], w[ts(grp * 4 + j, P), :])
                dstt = wts[grp]
            else:
                row = xld.tile([P, HIDDEN], F32, tag="xr", name=f"xr{grp}_{j}")
                nc.sync.dma_start(row[:], x[ts(grp * 4 + j, P), :])
                dstt = xts[grp]
            for kh in range(2):
                tpg = ps_g.tile([P, SETB, P], F32, tag="g")
                for k4 in range(SETB):
                    nc.tensor.transpose(tpg[:, k4, :],
                                        row[:, ts(kh * SETB + k4, P)],
                                        ident32[:])
                dst = dstt[:, ts(kh, SETB), ts(j, P)]
                if (j + kh) % 2 == 0:
                    cp(dst, tpg[:])
                else:
                    scp(dst, tpg[:])

        UNITS = ([("w", 0, j) for j in range(2)] + [("x", 0, 0), ("x", 0, 1)]
                 + [("w", 0, 2), ("w", 0, 3)] + [("x", 0, 2), ("x", 0, 3)]
                 + [("x", 1, j) for j in range(4)]
                 + [("w", 1, j) for j in range(4)])

        def emit_wT_xbar(og):
            wts[og] = wtsp.tile([P, NBLK, 512], MAIN_DT, tag="wts",
                                name=f"wts{og}")
            nc.sync.dma_start(wts[og][:], wb[ts(og, 512), :], transpose=True)

        def emit_xt_xbar(sg):
            t = xtp.tile([P, NBLK, 512], MAIN_DT, tag="xt", name=f"xt{sg}")
            nc.sync.dma_start(t[:], xb[ts(sg, 512), :], transpose=True)
            xts[sg] = t

        # ---------- Newton-Cayley: 6 interleaved sets of 4 blocks ----------
        p32_s, p16_s, x_s = [], [], []
        for s in range(NSETS):
            g = ps_g.tile([P, SETB, P], F32, tag="g")
            for j in range(SETB):                        # S^T @ S = -S^2
                nc.tensor.matmul(g[:, j, :], lhsT=s_s[s][:, j, :],
                                 rhs=s_s[s][:, j, :], start=True, stop=True)
            p32s = nper.tile([P, SETB, P], F32, tag=f"p32{s}", name=f"p32{s}")
            add(p32s[:], bc(eI2), g[:])                  # P = (1+e)^2 I - S^2
            p16s = nper.tile([P, SETB, P], F16, tag=f"p16{s}", name=f"p16{s}")
            scp(p16s[:], p32s[:])
            xset = nxp.tile([P, SETB, P], F16, tag=f"x{s}", name=f"x{s}_init")
            smul(xset[:], p32s[:], float(X0_B))          # X0 = aI + bP
            add(xset[:], xset[:], bc(aI0))
            # fold B^T = eI12 + (2+e)S - P into the S tile now
            nc.vector.tensor_scalar(s_s[s][:], s_s[s][:], float(2.0 + EPS),
                                    None, mybir.AluOpType.mult)
            add(s_s[s][:], s_s[s][:], bc(eI12))
            sub(s_s[s][:], s_s[s][:], p32s[:])
            p32_s.append(p32s)
            p16_s.append(p16s)
            x_s.append(xset)

        emit_unit(*UNITS[0])
        emit_unit(*UNITS[1])
        for i in range(NEWTON_F16):
            do_sym = i in SYM_ITERS
            for s in range(NSETS):
                # one PSUM tile per (iter, set): g1, then g2 overwrites it
                # (the DVE read of g1 gates the g2 matmuls anyway).
                g = ps_g.tile([P, SETB, P], F32, tag="g")
                for j in range(SETB):
                    nc.tensor.matmul(g[:, j, :], lhsT=p16_s[s][:, j, :],
                                     rhs=x_s[s][:, j, :], start=True,
                                     stop=True)
                u = nup.tile([P, SETB, P], F16, tag="u")
                sub(u[:], bc(twoI), g[:])                # U = 2I - P X (DVE)
                for j in range(SETB):                    # X' = X U
                    nc.tensor.matmul(g[:, j, :], lhsT=x_s[s][:, j, :],
                                     rhs=u[:, j, :], start=True, stop=True)
                xset = nxp.tile([P, SETB, P], F16, tag=f"x{s}",
                                name=f"x{s}_{i}")
                if not do_sym:
                    if s == 0:
                        cp(xset[:], g[:])                # DVE
                    else:
                        scp(xset[:], g[:])               # ScalarE
                else:
                    xc = misc.tile([P, SETB, P], F32, tag="xc")
                    cp(xc[:], g[:])
                    tpg = ps_g.tile([P, SETB, P], F32, tag="g")
                    for j in range(SETB):
                        nc.tensor.transpose(tpg[:, j, :], xc[:, j, :],
                                            ident32[:])
                    add(xc[:], xc[:], tpg[:])
                    nc.scalar.activation(xset[:], xc[:],
                                         mybir.ActivationFunctionType.Copy,
                                         scale=0.5)
                x_s[s] = xset
            emit_unit(*UNITS[2 + 2 * i])
            emit_unit(*UNITS[3 + 2 * i])
            if i == 4:
                # x^T sg0/sg1 complete -> release the deferred SWDGE casts
                # (keeps their 31 MB of HBM traffic out of the latency-
                # critical prelude window).
                trig = misc.tile([P, SETB], MAIN_DT, tag="trig")
                nc.gpsimd.partition_broadcast(trig[:], xts[1][:1, 0, :SETB])
                for og in range(2, O_TILES):
                    nc.gpsimd.dma_start(wb[ts(og, 512), :], w[ts(og, 512), :])
                for sg in range(2, SG):
                    nc.gpsimd.dma_start(xb[ts(sg, 512), :], x[ts(sg, 512), :])

        # fp32 polish + Q for a pair of sets (q first, then k, then v)
        q_s = [None] * NSETS

        def polish_q(pair):
            xfs = {}
            for s in pair:
                xf = nxp.tile([P, SETB, P], F32, tag=f"xf{s % 2}",
                              name=f"xf{s}")
                if s % 2 == 0:
                    cp(xf[:], x_s[s][:])
                else:
                    scp(xf[:], x_s[s][:])
                xfs[s] = xf
            for s in pair:
                g = ps_g.tile([P, SETB, P], F32, tag="g")
                for j in range(SETB):
                    nc.tensor.matmul(g[:, j, :], lhsT=p32_s[s][:, j, :],
                                     rhs=xfs[s][:, j, :], start=True,
                                     stop=True)
                uf = misc.tile([P, SETB, P], F32, tag=f"uf{s % 2}")
                sub(uf[:], bc(twoI), g[:])
                for j in range(SETB):
                    nc.tensor.matmul(g[:, j, :], lhsT=xfs[s][:, j, :],
                                     rhs=uf[:, j, :], start=True, stop=True)
                xf2 = nxp.tile([P, SETB, P], F32, tag=f"xf{s % 2}",
                               name=f"xf2_{s}")
                if s % 2 == 0:
                    cp(xf2[:], g[:])
                else:
                    scp(xf2[:], g[:])
                xfs[s] = xf2
            for s in pair:
                # Q = B @ X with B^T = eI12 + (2+e)S - P (folded into s_s)
                g = ps_g.tile([P, SETB, P], F32, tag="g")
                for j in range(SETB):
                    nc.tensor.matmul(g[:, j, :], lhsT=s_s[s][:, j, :],
                                     rhs=xfs[s][:, j, :], start=True,
                                     stop=True)
                qset = qpool.tile([P, SETB, P], MAIN_DT, tag=f"q{s}",
                                  name=f"q{s}")
                if s % 2 == 0:
                    cp(qset[:], g[:])
                else:
                    scp(qset[:], g[:])
                q_s[s] = qset

        def q_lhsT(n):
            return q_s[n // SETB][:, n % SETB, :]

        # ---- filtT chunks: ft[k][og] = Q^T W^T ----
        def emit_ft(og):
            part = og // 2             # q/k/v
            for k in range(NBLK):
                fg = ps_out.tile([P, 512], F32, tag="po", name=f"fg{og}_{k}")
                nc.tensor.matmul(fg[:], lhsT=q_lhsT(part * NBLK + k),
                                 rhs=wts[og][:, k, :], start=True, stop=True)
                if k % 2 == 0:
                    cp(ft[k][og][:], fg[:])
                else:
                    scp(ft[k][og][:], fg[:])

        # ---- main GEMM bursts ----
        def emit_gemm(mt, og_lo, n_og):
            sg, sb = mt // 4, mt % 4
            ob = obp.tile([P, 512 * n_og], F32, tag="ob",
                          name=f"ob{mt}_{og_lo}")
            for h in range(n_og):
                og = og_lo + h
                po = ps_out.tile([P, 512], F32, tag="po", name=f"po{mt}_{og}")
                for k in range(NBLK):
                    nc.tensor.matmul(po[:], lhsT=xts[sg][:, k, ts(sb, P)],
                                     rhs=ft[k][og][:],
                                     start=(k == 0), stop=(k == NBLK - 1))
                add(ob[:, ts(h, 512)], po[:], bias_bc[:, ts(og, 512)])
            nc.scalar.dma_start(
                out[ts(mt, P), ds(og_lo * 512, 512 * n_og)], ob[:])

        # ================= tail emission order (= priority) =============
        polish_q((0, 1))
        emit_ft(0)
        emit_ft(1)
        # phase 1: og0 then og1 over the first N1 row tiles; k/v polish and
        # og2-5 filtT builds slot into the stream between chunks.
        for mt in range(N1):
            emit_gemm(mt, 0, 1)
            if mt == 0:
                polish_q((2, 3))
            if mt == 2:
                emit_wT_xbar(2)
                emit_ft(2)
            if mt == 4:
                emit_wT_xbar(3)
                emit_ft(3)
        for mt in range(N1):
            emit_gemm(mt, 1, 1)
            if mt == 0:
                polish_q((4, 5))
            if mt == 2:
                emit_wT_xbar(4)
                emit_ft(4)
            if mt == 4:
                emit_wT_xbar(5)
                emit_ft(5)
        emit_xt_xbar(2)
        emit_xt_xbar(3)
        # phase 2: remaining row tiles, all og groups
        for mt in range(N1, M_TILES):
            if mt % 4 == 0 and mt + 8 < M_TILES:
                emit_xt_xbar((mt + 8) // 4)  # prefetch sg two groups ahead
            for pair in range(3):
                emit_gemm(mt, 2 * pair, 2)
        # phase 3: og2-5 for the first N1 row tiles (x^T still resident)
        for mt in range(N1):
            emit_gemm(mt, 2, 2)
            emit_gemm(mt, 4, 2)


def build():
    if "nc" in _CACHE:
        return _CACHE["nc"]
    import contextlib

    nc = bacc.Bacc("TRN2", target_bir_lowering=False, debug=False)
    with tile.TileContext(nc) as tc:
        with contextlib.ExitStack() as ctx:
            build_body(ctx, tc)
    nc.compile()
    _CACHE["nc"] = nc
    return nc


_CACHE = {}


def make_in_maps(attn_weight, bias, x, q_R, k_R, v_R):
    rmat = np.concatenate([q_R, k_R, v_R], axis=0).astype(np.float32)
    rmt = np.ascontiguousarray(rmat.transpose(1, 0, 2))  # [P, NROT, P]
    w = np.ascontiguousarray(attn_weight, dtype=np.float32)
    b = np.ascontiguousarray(bias, dtype=np.float32)
    return [
        {"x": np.ascontiguousarray(x[c], dtype=np.float32),
         "w": w, "bias": b, "rmt": rmt}
        for c in range(N_CORES)
    ]


def kernel(attn_weight, bias, x, q_R, k_R, v_R, **run_kwargs):
    nc = build()
    in_maps = make_in_maps(attn_weight, bias, x, q_R, k_R, v_R)
    res = run_bass_kernel_spmd(nc, in_maps, core_ids=list(range(N_CORES)),
                               **run_kwargs)
    out = np.stack([res.results[c]["out"] for c in range(N_CORES)], axis=0)
    _CACHE["last_results"] = res
    return out


# revision 15
# speedup vs baseline: 1.2044x; 1.0988x over previous
"""Trainium2 Bass kernel for EnhancedOFTQKVLayer.

Computes out[b,s,o] = x[b,s,:] @ filt[o,:]^T + bias[o] where
filt = [Wq @ BD(cayley(q_R)); Wk @ BD(cayley(k_R)); Wv @ BD(cayley(v_R))]
(BD = block-diagonal, cayley(A) = (I-S) inv(I+S+eps I), S = 0.5(A-A^T)).

Distribution: data-parallel - batch b (8 rows) sharded one per NeuronCore;
attn_weight / bias / rotation matrices replicated.

Per-core schedule (v6):
  1. ALL matmul-operand transposes are done by DMA, not compute engines:
     x and W are cast fp32->bf16 straight in DRAM by SWDGE cast-DMAs
     (gpsimd queue, ordered by deadline) and pulled into SBUF
     pre-transposed by 1 MB DMA-xbar loads.  Zero PE/DVE/ScalarE cost.
  2. Cayley via SPD Newton-Schulz on P = (1+eps)^2 I - S^2 (iterates are
     polynomials in S^2, hence symmetric -> lhsT=operand works without
     transposes).  X0 = aI + bP with (a, b) the true minimax-residual
     linear init on the measured spectrum [1, 254] - one fewer fp16
     iteration than the generic init for the same residual.  5 fp16
     iterations (symmetrize on 2 and 4) + 2 fp32 polish iterations; the
     polish runs pairwise q->k->v so the q-projection finishes first and
     the polish of k/v overlaps the phase-1 GEMM.  rmat is pre-permuted
     on the host so one contiguous DMA loads all rotation blocks.
  3. Main matmul in bf16 (fp32 PSUM), three phases: (1) og0 then og1 over
     the first 12 row tiles, interleaved with the k/v polish and the
     og2-5 filtT builds, (2) row-tile-outer over the remaining 20 tiles x
     all 6 og groups (x^T streams through a 3-slot ring), (3) og2-5 for
     the first 12 row tiles (pure GEMM tail, x^T sg0-2 still resident).
     Fused bias add on DVE, 512 KB output DMAs on the ACT queue.
"""

import numpy as np

import concourse.bass as bass
import concourse.mybir as mybir
import concourse.tile as tile
from concourse import bacc
from concourse.bass import ds, ts
from concourse.masks import make_identity
from concourse.bass_utils import run_bass_kernel_spmd

F32 = mybir.dt.float32
F16 = mybir.dt.float16
BF16 = mybir.dt.bfloat16

MAIN_DT = BF16           # dtype of the big matmul inputs (x, filtT)

HIDDEN = 1024
OUT_DIM = 3 * HIDDEN
SEQ = 4096
P = 128
NBLK = 8                 # 128-blocks per hidden
NROT = 24                # 3 * NBLK rotation blocks
EPS = 1e-6
N_CORES = 8

NSETS = 6                # Newton processes blocks in sets of 4
SETB = 4

# Newton-Schulz schedule (validated offline against the jax reference:
# max block rel-err 5.0e-3 vs 8.8e-3 for the older 7+1 schedule).
NEWTON_F16 = 5
NEWTON_F32 = 2
SYM_ITERS = {2, 4}       # symmetrize on these fp16 iterations
X0_A = 3.0874517e-02     # X0 = aI + bP (minimax residual on [1, 254])
X0_B = -1.2101700e-04

M_TILES = SEQ // P       # 32 row tiles of 128
SG = SEQ // 512          # 8 row groups of 512 (4 row tiles each)
O_TILES = OUT_DIM // 512  # 6
N1 = 12                  # phase-1 row tiles (og0/og1 early wave)


def build_body(ctx, tc):
    nc = tc.nc

    x = nc.dram_tensor("x", [SEQ, HIDDEN], F32, kind="ExternalInput").ap()
    w = nc.dram_tensor("w", [OUT_DIM, HIDDEN], F32, kind="ExternalInput").ap()
    bias = nc.dram_tensor("bias", [OUT_DIM], F32, kind="ExternalInput").ap()
    # host-side pre-permuted rotations: rmt[p, n, c] = rmat[n, p, c]
    rmt_d = nc.dram_tensor("rmt", [P, NROT, P], F32, kind="ExternalInput").ap()
    out = nc.dram_tensor("out", [SEQ, OUT_DIM], F32, kind="ExternalOutput").ap()

    sub = nc.vector.tensor_sub
    add = nc.vector.tensor_add
    smul = nc.vector.tensor_scalar_mul
    cp = nc.vector.tensor_copy
    scp = nc.scalar.copy

    def bc(t):  # broadcast a [P, P] constant over a set's middle dim
        return t[:].unsqueeze(1).to_broadcast([P, SETB, P])

    # ---- persistent pools ----
    const = ctx.enter_context(tc.tile_pool(name="const", bufs=1))
    ftp = ctx.enter_context(tc.tile_pool(name="ftp", bufs=1))
    qpool = ctx.enter_context(tc.tile_pool(name="qpool", bufs=1))
    dram = ctx.enter_context(tc.tile_pool(name="dram", bufs=1, space="DRAM"))

    ident32 = const.tile([P, P], F32)
    make_identity(nc, ident32)
    eI2 = const.tile([P, P], F32)       # (1+eps)^2 I
    smul(eI2[:], ident32[:], float((1.0 + EPS) ** 2))
    eI12 = const.tile([P, P], F32)      # ((1+eps) + (1+eps)^2) I
    smul(eI12[:], ident32[:], float((1.0 + EPS) + (1.0 + EPS) ** 2))
    twoI = const.tile([P, P], F32)      # 2 I
    smul(twoI[:], ident32[:], 2.0)
    aI0 = const.tile([P, P], F16)       # X0_A * I  (Newton init)
    smul(aI0[:], ident32[:], float(X0_A))

    # bf16 DRAM scratch (filled by SWDGE cast-DMAs, deadline order)
    xb = dram.tile([SEQ, HIDDEN], MAIN_DT)
    wb = dram.tile([OUT_DIM, HIDDEN], MAIN_DT)

    nc.gpsimd.dma_start(wb[ts(0, 512), :], w[ts(0, 512), :])
    nc.gpsimd.dma_start(xb[ts(0, 512), :], x[ts(0, 512), :])
    nc.gpsimd.dma_start(wb[ts(1, 512), :], w[ts(1, 512), :])
    nc.gpsimd.dma_start(xb[ts(1, 512), :], x[ts(1, 512), :])

    bias_bc = const.tile([P, OUT_DIM], MAIN_DT)
    with tc.tile_pool(name="biasld", bufs=1) as bl:
        brow = bl.tile([1, OUT_DIM], F32)
        nc.sync.dma_start(brow[:], bias.unsqueeze(0))
        cp(bias_bc[:1, :], brow[:])
    nc.gpsimd.partition_broadcast(bias_bc[:], bias_bc[:1, :])

    for og in range(2, O_TILES):
        nc.gpsimd.dma_start(wb[ts(og, 512), :], w[ts(og, 512), :])
    for sg in range(2, SG):
        nc.gpsimd.dma_start(xb[ts(sg, 512), :], x[ts(sg, 512), :])

    # filtT chunks: ft[k][og][c, o'] = filtT[k*128+c, og*512+o']
    ft = [[ftp.tile([P, 512], MAIN_DT, tag=f"ft{k}_{og}", name=f"ft{k}_{og}")
           for og in range(O_TILES)] for k in range(NBLK)]

    with (
        tc.tile_pool(name="nper", bufs=1) as nper,     # per-set persistents
        tc.tile_pool(name="nx", bufs=1) as nxp,        # per-set iterates
        tc.tile_pool(name="nu", bufs=3) as nup,        # U temp
        tc.tile_pool(name="misc", bufs=1) as misc,
        tc.tile_pool(name="wtsp", bufs=2) as wtsp,     # W^T og ring
        tc.tile_pool(name="xt01", bufs=1) as xt01p,    # x^T sg0-2 resident
        tc.tile_pool(name="xtp", bufs=2) as xtp,       # x^T sg3-7 ring
        tc.tile_pool(name="obp", bufs=3) as obp,       # out staging
        tc.tile_pool(name="ps_g", bufs=4, space="PSUM") as ps_g,
        tc.tile_pool(name="ps_out", bufs=4, space="PSUM") as ps_out,
    ):
        # ---- W^T / x^T via DMA-xbar transposed loads ----
        wts = {}

        def emit_wT(og):
            wts[og] = wtsp.tile([P, NBLK, 512], MAIN_DT, tag="wts",
                                name=f"wts{og}")
            nc.sync.dma_start(wts[og][:], wb[ts(og, 512), :], transpose=True)

        xts = {}

        def emit_xt(sg):
            pool, tag = (xt01p, f"xt{sg}") if sg < 3 else (xtp, "xt")
            t = pool.tile([P, NBLK, 512], MAIN_DT, tag=tag, name=f"xt{sg}")
            nc.sync.dma_start(t[:], xb[ts(sg, 512), :], transpose=True)
            xts[sg] = t

        # ------- rotations + S prep (rmt freed right after) -------
        s_s = []
        with tc.tile_pool(name="rmt", bufs=1) as rmtp:
            rmt = rmtp.tile([P, NROT, P], F32)
            nc.sync.dma_start(rmt[:], rmt_d)
            emit_wT(0)
            emit_xt(0)
            emit_wT(1)
            emit_xt(1)
            for s in range(NSETS):
                tpg = ps_g.tile([P, SETB, P], F32, tag="g")
                for j in range(SETB):
                    nc.tensor.transpose(tpg[:, j, :], rmt[:, s * SETB + j, :],
                                        ident32[:])
                sset = nper.tile([P, SETB, P], F32, tag=f"s{s}", name=f"s{s}")
                for j in range(SETB):
                    sub(sset[:, j, :], rmt[:, s * SETB + j, :], tpg[:, j, :])
                smul(sset[:], sset[:], 0.5)              # S
                s_s.append(sset)

        # ---------- Newton-Cayley: 6 interleaved sets of 4 blocks ----------
        p32_s, p16_s, x_s = [], [], []
        for s in range(NSETS):
            g = ps_g.tile([P, SETB, P], F32, tag="g")
            for j in range(SETB):                        # S^T @ S = -S^2
                nc.tensor.matmul(g[:, j, :], lhsT=s_s[s][:, j, :],
                                 rhs=s_s[s][:, j, :], start=True, stop=True)
            p32s = nper.tile([P, SETB, P], F32, tag=f"p32{s}", name=f"p32{s}")
            add(p32s[:], bc(eI2), g[:])                  # P = (1+e)^2 I - S^2
            p16s = nper.tile([P, SETB, P], F16, tag=f"p16{s}", name=f"p16{s}")
            scp(p16s[:], p32s[:])
            xset = nxp.tile([P, SETB, P], F16, tag=f"x{s}", name=f"x{s}_init")
            smul(xset[:], p32s[:], float(X0_B))          # X0 = aI + bP
            add(xset[:], xset[:], bc(aI0))
            # fold B^T = eI12 + (2+e)S - P into the S tile now
            nc.vector.tensor_scalar(s_s[s][:], s_s[s][:], float(2.0 + EPS),
                                    None, mybir.AluOpType.mult)
            add(s_s[s][:], s_s[s][:], bc(eI12))
            sub(s_s[s][:], s_s[s][:], p32s[:])
            p32_s.append(p32s)
            p16_s.append(p16s)
            x_s.append(xset)

        for i in range(NEWTON_F16):
            do_sym = i in SYM_ITERS
            for s in range(NSETS):
                # one PSUM tile per (iter, set): g1, then g2 overwrites it
                # (the DVE read of g1 gates the g2 matmuls anyway).
                g = ps_g.tile([P, SETB, P], F32, tag="g")
                for j in range(SETB):
                    nc.tensor.matmul(g[:, j, :], lhsT=p16_s[s][:, j, :],
                                     rhs=x_s[s][:, j, :], start=True,
                                     stop=True)
                u = nup.tile([P, SETB, P], F16, tag="u")
                sub(u[:], bc(twoI), g[:])                # U = 2I - P X (DVE)
                for j in range(SETB):                    # X' = X U
                    nc.tensor.matmul(g[:, j, :], lhsT=x_s[s][:, j, :],
                                     rhs=u[:, j, :], start=True, stop=True)
                xset = nxp.tile([P, SETB, P], F16, tag=f"x{s}",
                                name=f"x{s}_{i}")
                if not do_sym:
                    if s == 0:
                        cp(xset[:], g[:])                # DVE
                    else:
                        scp(xset[:], g[:])               # ScalarE
                else:
                    xc = misc.tile([P, SETB, P], F32, tag="xc")
                    cp(xc[:], g[:])
                    tpg = ps_g.tile([P, SETB, P], F32, tag="g")
                    for j in range(SETB):
                        nc.tensor.transpose(tpg[:, j, :], xc[:, j, :],
                                            ident32[:])
                    add(xc[:], xc[:], tpg[:])
                    nc.scalar.activation(xset[:], xc[:],
                                         mybir.ActivationFunctionType.Copy,
                                         scale=0.5)
                x_s[s] = xset

        # fp32 polish + Q for a pair of sets (q first, then k, then v)
        q_s = [None] * NSETS

        def polish_q(pair):
            xfs = {}
            for s in pair:
                xf = nxp.tile([P, SETB, P], F32, tag=f"xf{s % 2}",
                              name=f"xf{s}")
                if s % 2 == 0:
                    cp(xf[:], x_s[s][:])
                else:
                    scp(xf[:], x_s[s][:])
                xfs[s] = xf
            for i in range(NEWTON_F32):
                for s in pair:
                    g = ps_g.tile([P, SETB, P], F32, tag="g")
                    for j in range(SETB):
                        nc.tensor.matmul(g[:, j, :], lhsT=p32_s[s][:, j, :],
                                         rhs=xfs[s][:, j, :], start=True,
                                         stop=True)
                    uf = misc.tile([P, SETB, P], F32, tag=f"uf{s % 2}")
                    sub(uf[:], bc(twoI), g[:])
                    for j in range(SETB):
                        nc.tensor.matmul(g[:, j, :], lhsT=xfs[s][:, j, :],
                                         rhs=uf[:, j, :], start=True,
                                         stop=True)
                    xf2 = nxp.tile([P, SETB, P], F32, tag=f"xf{s % 2}",
                                   name=f"xf{s}_{i}")
                    if s % 2 == 0:
                        cp(xf2[:], g[:])
                    else:
                        scp(xf2[:], g[:])
                    xfs[s] = xf2
            for s in pair:
                # Q = B @ X with B^T = eI12 + (2+e)S - P (folded into s_s)
                g = ps_g.tile([P, SETB, P], F32, tag="g")
                for j in range(SETB):
                    nc.tensor.matmul(g[:, j, :], lhsT=s_s[s][:, j, :],
                                     rhs=xfs[s][:, j, :], start=True,
                                     stop=True)
                qset = qpool.tile([P, SETB, P], MAIN_DT, tag=f"q{s}",
                                  name=f"q{s}")
                if s % 2 == 0:
                    cp(qset[:], g[:])
                else:
                    scp(qset[:], g[:])
                q_s[s] = qset

        def q_lhsT(n):
            return q_s[n // SETB][:, n % SETB, :]

        # ---- filtT chunks: ft[k][og] = Q^T W^T ----
        def emit_ft(og):
            part = og // 2             # q/k/v
            for k in range(NBLK):
                fg = ps_out.tile([P, 512], F32, tag="po", name=f"fg{og}_{k}")
                nc.tensor.matmul(fg[:], lhsT=q_lhsT(part * NBLK + k),
                                 rhs=wts[og][:, k, :], start=True, stop=True)
                if k % 2 == 0:
                    cp(ft[k][og][:], fg[:])
                else:
                    scp(ft[k][og][:], fg[:])

        # ---- main GEMM bursts ----
        def emit_gemm(mt, og_lo, n_og):
            sg, sb = mt // 4, mt % 4
            ob = obp.tile([P, 512 * n_og], F32, tag="ob",
                          name=f"ob{mt}_{og_lo}")
            for h in range(n_og):
                og = og_lo + h
                po = ps_out.tile([P, 512], F32, tag="po", name=f"po{mt}_{og}")
                for k in range(NBLK):
                    nc.tensor.matmul(po[:], lhsT=xts[sg][:, k, ts(sb, P)],
                                     rhs=ft[k][og][:],
                                     start=(k == 0), stop=(k == NBLK - 1))
                add(ob[:, ts(h, 512)], po[:], bias_bc[:, ts(og, 512)])
            nc.scalar.dma_start(
                out[ts(mt, P), ds(og_lo * 512, 512 * n_og)], ob[:])

        # ================= tail emission order (= priority) =============
        polish_q((0, 1))
        emit_ft(0)
        emit_ft(1)
        emit_xt(2)
        # phase 1: og0 then og1 over the first N1 row tiles; k/v polish and
        # og2-5 filtT builds slot into the stream between chunks.
        for mt in range(N1):
            emit_gemm(mt, 0, 1)
            if mt == 0:
                polish_q((2, 3))
            if mt == 2:
                emit_wT(2)
                emit_ft(2)
            if mt == 5:
                emit_wT(3)
                emit_ft(3)
        emit_xt(3)
        for mt in range(N1):
            emit_gemm(mt, 1, 1)
            if mt == 0:
                polish_q((4, 5))
            if mt == 2:
                emit_wT(4)
                emit_ft(4)
            if mt == 5:
                emit_wT(5)
                emit_ft(5)
        emit_xt(4)
        # phase 2: remaining row tiles, all og groups
        for mt in range(N1, M_TILES):
            if mt % 4 == 0 and mt + 8 < M_TILES:
                emit_xt((mt + 8) // 4)      # prefetch sg two groups ahead
            for pair in range(3):
                emit_gemm(mt, 2 * pair, 2)
        # phase 3: og2-5 for the first N1 row tiles (x^T still resident)
        for mt in range(N1):
            emit_gemm(mt, 2, 2)
            emit_gemm(mt, 4, 2)


def build():
    if "nc" in _CACHE:
        return _CACHE["nc"]
    import contextlib

    nc = bacc.Bacc("TRN2", target_bir_lowering=False, debug=False)
    with tile.TileContext(nc) as tc:
        with contextlib.ExitStack() as ctx:
            build_body(ctx, tc)
    nc.compile()
    _CACHE["nc"] = nc
    return nc


_CACHE = {}


def make_in_maps(attn_weight, bias, x, q_R, k_R, v_R):
    rmat = np.concatenate([q_R, k_R, v_R], axis=0).astype(np.float32)
    rmt = np.ascontiguousarray(rmat.transpose(1, 0, 2))  # [P, NROT, P]
    w = np.ascontiguousarray(attn_weight, dtype=np.float32)
    b = np.ascontiguousarray(bias, dtype=np.float32)
    return [
        {"x": np.ascontiguousarray(x[c], dtype=np.float32),
         "w": w, "bias": b, "rmt": rmt}
        for c in range(N_CORES)
    ]


def kernel(attn_weight, bias, x, q_R, k_R, v_R, **run_kwargs):
    nc = build()
    in_maps = make_in_maps(attn_weight, bias, x, q_R, k_R, v_R)
    res = run_bass_kernel_spmd(nc, in_maps, core_ids=list(range(N_CORES)),
                               **run_kwargs)
    out = np.stack([res.results[c]["out"] for c in range(N_CORES)], axis=0)
    _CACHE["last_results"] = res
    return out
